# revision 1
# baseline (speedup 1.0000x reference)
"""Trainium2 Bass kernel for nn_Block_51178830299350 (dense transformer block).

Strategy (8 NeuronCores, single NEFF):
  - Head-tensor-parallel attention: 2 heads/core over all 4096 tokens, bf16
    matmuls everywhere (fp8 DoubleRow paths exist behind K_*_F8 env flags but
    cost too much accuracy for the 2e-2 gate to enable by default).
  - LN1: per-core stats on own 512 tokens + tiny AllGather of rsqrt row.
    Mean-subtraction folded into pre-packed qkv weights (host side).
  - RoPE: head-dims permuted host-side into (j, j+32) adjacent pairs so the
    half-swap is a single in-quadrant stream_shuffle; LN scale (rs) folded
    into per-tile cos/sin tables; V's rs scale fused into its PSUM-drain mul.
  - Softmax normalization deferred: unnormalized PV + denominator row ride
    the AllToAll; one reciprocal + select-matmul broadcast normalizes after.
  - head-1 attention for jt<3 (plus (1,b0,3)) interleaves into the qkv t-loop
    to fill otherwise Act-bound windows; (1,b1,3) stays in ph2 to hide a2a0.
  - LN2 mean/var broadcast across partitions for free via the ones-matmul.
  - AllToAll (2x ~0.5MB/rank) redistributes head-sharded attention output to
    token-sharded; proj + FFN run token-parallel with full weights streamed.
"""
import sys

sys.path.insert(0, "/opt/trn_rl_repo")

import numpy as np

B, S, D, H, HD = 2, 2048, 1024, 16, 64
INNER = 4 * D
NCORES = 8
TOK = B * S              # 4096 flat tokens
TOWN = TOK // NCORES     # 512 tokens per core
EPS = 1e-5
WSCALE = 64.0            # fp8 weight pre-scale (2^6)
_BUILD_CACHE = {}

# dtype knobs
import os as _os
QKV_F8 = _os.environ.get("K_QKV_F8", "0") == "1"
PROJ_F8 = _os.environ.get("K_PROJ_F8", "0") == "1"
FC_F8 = _os.environ.get("K_FC_F8", "0") == "1"
PV_F8 = _os.environ.get("K_PV_F8", "0") == "1"
SPLITPROJ = _os.environ.get("K_SPLITPROJ", "0") == "1"


def round_fp32r(x: np.ndarray) -> np.ndarray:
    """Round fp32 -> fp32r (11-bit mantissa, RTNE), matching TRN2 PE input fmt."""
    v = np.ascontiguousarray(x, dtype=np.float32).view(np.uint32)
    low = v & np.uint32(0xFFF)
    half = np.uint32(0x800)
    rounded = (v & ~np.uint32(0xFFF)).copy()
    up = (low > half) | ((low == half) & (((v >> np.uint32(12)) & np.uint32(1)) != 0))
    rounded[up] += np.uint32(0x1000)
    return rounded.view(np.float32)


def _np_reference(x, mask, sin, cos, ln1_w, ln1_b, w_qkv, w_proj, ln2_w, ln2_b,
                  w_fc1, w_fc2):
    """Slow numpy fallback (only used if inputs violate kernel assumptions)."""
    from scipy.special import erf

    def ln(t, w, b):
        m = t.mean(-1, keepdims=True)
        v = ((t - m) ** 2).mean(-1, keepdims=True)
        return (t - m) / np.sqrt(v + EPS) * w + b

    def rope(t, sin, cos):
        half = t.shape[-1] // 2
        rot = np.concatenate([-t[..., half:], t[..., :half]], -1)
        return t * cos + rot * sin

    b, s, d = x.shape
    hx = ln(x, ln1_w, ln1_b)
    qkv = (hx @ w_qkv).reshape(b, s, 3, H, HD).transpose(2, 0, 3, 1, 4)
    q, k, v = qkv[0], qkv[1], qkv[2]
    q = rope(q, sin, cos)
    k = rope(k, sin, cos)
    att = np.einsum("bhqd,bhkd->bhqk", q, k) / np.sqrt(HD)
    att = np.where(mask, att, -np.inf)
    att = att - att.max(-1, keepdims=True)
    p = np.exp(att)
    p /= p.sum(-1, keepdims=True)
    o = np.einsum("bhqk,bhkd->bhqd", p, v)
    o = o.transpose(0, 2, 1, 3).reshape(b, s, d) @ w_proj
    x = x + o
    h2 = ln(x, ln2_w, ln2_b)
    h2 = h2 @ w_fc1
    h2 = 0.5 * h2 * (1.0 + erf(h2 / np.sqrt(2.0)))
    h2 = h2 @ w_fc2
    return (x + h2).astype(np.float32)


def _build(debug=False, sim=False):
    key = ("nc", debug, sim)
    if key in _BUILD_CACHE:
        return _BUILD_CACHE[key]
    import concourse.bacc as bacc
    import concourse.bass as bass_mod
    import concourse.tile as tile
    from concourse import mybir

    F32 = mybir.dt.float32
    F32R = mybir.dt.float32r
    BF16 = mybir.dt.bfloat16
    F8 = mybir.dt.float8e4
    AF = mybir.ActivationFunctionType
    DR = mybir.MatmulPerfMode.DoubleRow

    W8 = F8 if QKV_F8 else BF16       # qkv weight/x dtype
    WP8 = F8 if PROJ_F8 else BF16     # proj weight/attn dtype
    WF8 = F8 if FC_F8 else BF16       # fc weight/act dtype
    PT8 = F8 if PV_F8 else BF16       # probs/v dtype

    nc = bacc.Bacc("TRN2", target_bir_lowering=False, debug=False,
                   enable_asserts=False, num_devices=NCORES)

    # ---------------- DRAM parameters (per core) ----------------
    # packed transposed x: [128, t*4096 + dt*512 + i]
    xtp = nc.dram_tensor("xtp", [128, 8 * 8 * 512], W8, kind="ExternalInput")
    # own-slice xT (pre-rounded) for stats + residual: [128, dt*512 + i]
    xres = nc.dram_tensor("xres", [128, 8 * 512], F32R, kind="ExternalInput")
    # qkv weights for this core's 2 heads: [128, dt*384 + {Q(128)|K(128)|V(128)}]
    wqkv = nc.dram_tensor("wqkv", [128, 8 * 384], W8, kind="ExternalInput")
    # proj weights (full): [128, (jb*8+dt)*128 + j]
    wproj = nc.dram_tensor("wproj", [128, 8 * 8 * 128], WP8, kind="ExternalInput")
    # fc1 (full, ln2_w folded): [128, (j*8+dt)*128 + jj]
    wfc1 = nc.dram_tensor("wfc1", [128, 32 * 8 * 128], WF8, kind="ExternalInput")
    # fc2 (full): [128, (d*32+jt)*128 + dd]
    wfc2 = nc.dram_tensor("wfc2", [128, 8 * 32 * 128], WF8, kind="ExternalInput")
    # rope tables [128, S]: pair-interleaved head-dim order, stacked for both
    # local heads, sign-folded sin, 1/WSCALE descale folded when QKV_F8
    cosr = nc.dram_tensor("cosr", [128, S], BF16, kind="ExternalInput")
    sinr = nc.dram_tensor("sinr", [128, S], BF16, kind="ExternalInput")
    # 4 canonical diagonal mask tiles [128k, 512q]: 0/1 multiplicative, or
    # 0/-60 additive score bias when PV_F8
    maskt = nc.dram_tensor("maskt", [128, 4 * 512], BF16, kind="ExternalInput")
    id128 = nc.dram_tensor("id128", [128, 128], BF16, kind="ExternalInput")
    # constants
    ones128 = nc.dram_tensor("ones128", [128, 128], F32R, kind="ExternalInput")  # 1/1024
    ident2 = nc.dram_tensor("ident2", [128, 64], BF16, kind="ExternalInput")     # eye64 x2
    # denom broadcast one-hots [8, dt*128 + p]: selt[r, dt*128+p] = (r == dt)
    selt = nc.dram_tensor("selt", [8, 8 * 128], F32R, kind="ExternalInput")

    outT = nc.dram_tensor("outT", [128, 8 * 512], F32, kind="ExternalOutput")

    # collective bounce buffers
    ag_in = nc.dram_tensor("ag_in", [1, TOWN], F32)
    ag_out = nc.dram_tensor("ag_out", [NCORES, TOWN], F32, addr_space="Shared")
    a2a_in = [nc.dram_tensor(f"a2a_in{h}", [NCORES, 65, TOWN], BF16)
              for h in range(2)]
    a2a_out = [nc.dram_tensor(f"a2a_out{h}", [NCORES, 65, TOWN], BF16)
               for h in range(2)]

    dbg = {}
    if debug:
        for name, shape in [("dbg_q", [128, TOK]),
                            ("dbg_k", [128, TOK]), ("dbg_vtok", [128, 16 * 65]),
                            ("dbg_rs", [1, TOK]), ("dbg_pt", [128, 512]),
                            ("dbg_att", [128, 8 * 512]), ("dbg_x2", [128, 8 * 512]),
                            ("dbg_h1", [128, 4 * 512]), ("dbg_rs2", [1, TOWN]),
                            ("dbg_m2", [1, TOWN])]:
            dbg[name] = nc.dram_tensor(name, shape, F32, kind="ExternalOutput")

    RG = [list(range(NCORES))]
    SHUF_PAIR = [i ^ 1 for i in range(32)]

    def bc_ap(dram, nparts, ncols, offset=0):
        """partition-broadcast read AP over a DRAM row."""
        return bass_mod.AP(tensor=dram.ap().tensor, offset=offset,
                           ap=[[0, nparts], [1, ncols]])

    with tile.TileContext(nc) as tc:
        import contextlib
        with contextlib.ExitStack() as ctx:
            consts = ctx.enter_context(tc.tile_pool(name="consts", bufs=1))
            xrpool = ctx.enter_context(tc.tile_pool(name="xrpool", bufs=1))
            qkctx = contextlib.ExitStack()
            qkpool = qkctx.enter_context(tc.tile_pool(name="qk", bufs=1))
            vpool = qkctx.enter_context(tc.tile_pool(name="vtok", bufs=1))
            attsb = qkctx.enter_context(tc.tile_pool(name="attsb", bufs=2))
            attps = qkctx.enter_context(
                tc.tile_pool(name="attps", bufs=2, space="PSUM"))
            ph1ps_ctx = contextlib.ExitStack()
            ph1ps = ph1ps_ctx.enter_context(
                tc.tile_pool(name="ph1ps", bufs=2, space="PSUM"))

            ones_t = consts.tile([128, 128], F32R)
            nc.sync.dma_start(out=ones_t[:], in_=ones128[:, :])
            mask_t = consts.tile([128, 4, 512], BF16)
            nc.sync.dma_start(out=mask_t[:],
                              in_=maskt[:, :].rearrange("p (o q) -> p o q", o=4))
            id128_t = consts.tile([128, 128], BF16)
            if PV_F8:
                nc.sync.dma_start(out=id128_t[:], in_=id128[:, :])
            eps_t = consts.tile([1, 1], F32)
            nc.vector.memset(eps_t[:], EPS)
            eps128 = consts.tile([128, 1], F32)
            nc.vector.memset(eps128[:], EPS)
            # PE P-state warm-up: ~3.5us of throwaway matmuls so the first
            # real matmuls (LN1 stats, qkv t0) run at full clock
            wrm = consts.tile([128, 128], BF16)
            nc.vector.memset(wrm[:], 0.0)
            wrm2 = consts.tile([128, 512], BF16)
            nc.vector.memset(wrm2[:], 0.0)
            for _ in range(8):
                wps = attps.tile([128, 512], F32, tag="ps", name="wps", bufs=3)
                nc.tensor.matmul(wps[:], wrm[:], wrm2[:], start=True, stop=True)
            # fc1 weights prefetched fully when fp8 (32KB/partition); streamed
            # per-chunk in the fc1 loop otherwise (bf16 would be 64KB)
            wf1_t = None
            if FC_F8:
                wf1_t = consts.tile([128, 32, 8, 128], WF8)
                for g in range(4):
                    nc.sync.dma_start(
                        out=wf1_t[:, g * 8:(g + 1) * 8, :, :],
                        in_=wfc1[:, g * 8192:(g + 1) * 8192]
                        .rearrange("p (j d c) -> p j d c", j=8, d=8))
            wpj_t = consts.tile([128, 8, 8, 128], WP8)
            sel_t = consts.tile([8, 8, 128], F32R)
            nc.sync.dma_start(out=sel_t[:],
                              in_=selt[:, :].rearrange("p (d c) -> p d c", d=8))

            # persistent Q', K' (feature-major, 2 local heads x 64 dims)
            q_t = qkpool.tile([128, TOK], BF16)
            k_t = qkpool.tile([128, TOK], BF16)
            # [128, kt-pair, pair-elem, 65]: fp8 DoubleRow ldweights wants the
            # two weight rows contiguous (col 64 = ones for the denominator)
            VW = 128 if PV_F8 else 65  # fp8 dual ldweights wants 128 rows
            v_tok = [[vpool.tile([128, 8, 2, VW], PT8, tag=f"vtok{b}{h}",
                                 name=f"vtok{b}{h}")
                      for h in range(2)] for b in range(B)]

            def attn_jt(hl, b, jt, kt_lo=0, kt_hi=None, po_ext=None):
                """PV accumulation for q-tile jt over k-tiles [kt_lo, kt_hi).
                po_ext carries a partially-accumulated PSUM tile across split
                calls; the a2a store is emitted when kt_hi reaches the end."""
                o = hl * 64
                nkt = 4 * (jt + 1)
                if kt_hi is None:
                    kt_hi = nkt
                if po_ext is not None:
                    po = po_ext
                else:
                    po = attps.tile([128 if PV_F8 else 65, 512], F32, tag="po",
                                    name="po", bufs=2)
                qs = q_t[o:o + 64, b * S + jt * 512: b * S + (jt + 1) * 512]
                if PV_F8:
                    for kp in range(kt_lo // 2, kt_hi // 2):
                        pt = attsb.tile([128, 2, 512], F8, tag="pt", name="pt",
                                        bufs=4)
                        for ki in range(2):
                            kt = kp * 2 + ki
                            ps = attps.tile([128, 512], F32, tag="ps", name="ps",
                                            bufs=3)
                            ks = k_t[o:o + 64,
                                     b * S + kt * 128: b * S + (kt + 1) * 128]
                            od = kt - 4 * jt
                            if od >= 0:
                                # causal mask folded in as additive -60 bias
                                nc.tensor.matmul(ps[:], ks, qs, start=True,
                                                 stop=False)
                                nc.tensor.matmul(ps[:], id128_t[:],
                                                 mask_t[:, od, :],
                                                 start=False, stop=True)
                            else:
                                nc.tensor.matmul(ps[:], ks, qs, start=True,
                                                 stop=True)
                            nc.scalar.activation(pt[:, ki, :], ps[:], AF.Exp)
                        nc.tensor.matmul(
                            po[:], v_tok[b][hl][:, kp, :, :],
                            pt[:], start=(kp == 0), stop=(kp == nkt // 2 - 1),
                            perf_mode=DR)
                else:
                    for kt in range(kt_lo, kt_hi):
                        od = kt - 4 * jt
                        # diagonal-band tiles: columns q < od*128 are fully
                        # masked, so compute only the valid q-range
                        q0 = max(0, od) * 128
                        qn = 512 - q0
                        ps = attps.tile([128, 512], F32, tag="ps", name="ps",
                                        bufs=3)
                        ks = k_t[o:o + 64,
                                 b * S + kt * 128: b * S + (kt + 1) * 128]
                        nc.tensor.matmul(ps[:, 0:qn], ks, qs[:, q0:512],
                                         start=True, stop=True)
                        pt = attsb.tile([128, 512], BF16, tag="pt", name="pt",
                                        bufs=4)
                        nc.scalar.activation(pt[:, 0:qn], ps[:, 0:qn], AF.Exp)
                        if od >= 0:
                            nc.vector.tensor_mul(pt[:, 0:qn], pt[:, 0:qn],
                                                 mask_t[:, od, q0:512])
                        if debug and b == 0 and hl == 0 and jt == 0 and kt == 0:
                            dbf = attsb.tile([128, 512], F32, tag="dbf")
                            nc.vector.tensor_copy(dbf[:], pt[:])
                            nc.sync.dma_start(out=dbg["dbg_pt"][:, :], in_=dbf[:])
                        nc.tensor.matmul(po[:, q0:512],
                                         v_tok[b][hl][:, kt // 2, kt % 2, :],
                                         pt[:, 0:qn],
                                         start=(kt == 0), stop=(kt == nkt - 1),
                                         skip_group_check=True)
                if kt_hi < nkt:
                    return po
                ov = attsb.tile([65, 512], BF16, tag="ov", name="ov")
                nc.vector.tensor_copy(ov[:], po[0:65, :])
                nc.sync.dma_start(out=a2a_in[hl][b * 4 + jt, :, :], in_=ov[:])

            xown = xrpool.tile([128, 8, 512], F32R)
            # ================= phase 1a: LN1 stats + AllGather =================
            with nc.named_scope("ph1a_ln1"), tc.tile_pool(name="ph1a", bufs=1) as ph1a:
                nc.sync.dma_start(out=xown[:],
                                  in_=xres[:, :].rearrange("p (d c) -> p d c", d=8))
                sq = ph1a.tile([128, 8, 512], F32R)
                mps = attps.tile([128, 512], F32, tag="po", name="mps", bufs=2)
                sps = attps.tile([128, 512], F32, tag="po", name="sps", bufs=2)
                for dt in range(8):
                    nc.scalar.square(sq[:, dt, :], xown.bitcast(F32)[:, dt, :])
                for dt in range(8):
                    nc.tensor.matmul(mps[:], ones_t[:], xown[:, dt, :],
                                     start=(dt == 0), stop=(dt == 7))
                for dt in range(8):
                    nc.tensor.matmul(sps[:], ones_t[:], sq[:, dt, :],
                                     start=(dt == 0), stop=(dt == 7))
                mrow = ph1a.tile([1, 512], F32)
                nc.vector.tensor_copy(mrow[:], mps[0:1, :])
                msq = ph1a.tile([1, 512], F32)
                nc.vector.tensor_copy(msq[:], sps[0:1, :])
                var = ph1a.tile([1, 512], F32)
                nc.vector.tensor_mul(var[:], mrow[:], mrow[:])
                nc.vector.tensor_sub(var[:], msq[:], var[:])
                rsq = ph1a.tile([1, 512], F32)
                nc.scalar.activation(rsq[:], var[:], AF.Sqrt, bias=eps_t[:])
                rs_own = ph1a.tile([1, 512], F32)
                nc.vector.reciprocal(rs_own[:], rsq[:])
                nc.sync.dma_start(out=ag_in[:, :], in_=rs_own[:])
                if sim:
                    nc.sync.dma_start(out=ag_out[:, :],
                                      in_=bc_ap(ag_in, NCORES, TOWN))
                else:
                    nc.gpsimd.collective_compute(
                        "AllGather", mybir.AluOpType.bypass, replica_groups=RG,
                        ins=[ag_in.ap().opt()], outs=[ag_out.ap().opt()])
                if debug:
                    nc.sync.dma_start(
                        out=dbg["dbg_rs"][:, :],
                        in_=ag_out[:, :].rearrange("r t -> (r t)")
                        .rearrange("(o t) -> o t", o=1))

            # ====== phase 1b: qkv matmuls + RoPE + V fixup, per 512-token tile =
            with nc.named_scope("ph1b_qkv_attn0"), \
                 tc.tile_pool(name="ph1b", bufs=1) as ph1b, \
                 tc.tile_pool(name="ph1x", bufs=2) as ph1x:
                id_t = ph1b.tile([128, 64], BF16)
                nc.sync.dma_start(out=id_t[:], in_=ident2[:, :])
                wq_t = ph1b.tile([128, 8, 384], W8)
                nc.sync.dma_start(out=wq_t[:],
                                  in_=wqkv[:, :].rearrange("p (d c) -> p d c", d=8))
                cos_t = ph1b.tile([128, S], BF16)
                sin_t = ph1b.tile([128, S], BF16)
                rs_full = ph1b.tile([128, 8, 512], F32)
                for b in range(B):
                    for hl in range(2):
                        nc.vector.memset(v_tok[b][hl][:, :, :, 64:65], 1.0)
                        if PV_F8:
                            nc.vector.memset(v_tok[b][hl][:, :, :, 65:], 0.0)

                for t in range(8):
                    b, soff = t // 4, (t % 4) * 512
                    tsl = slice(t * 512, (t + 1) * 512)
                    ssl = slice(soff, soff + 512)
                    xt = ph1x.tile([128, 8, 512], W8, tag="xt", bufs=3)
                    if t == 0:
                        # cold start: per-dt chunks let qkv dt=0 begin after
                        # 1/8 of the transfer
                        for dt in range(8):
                            nc.sync.dma_start(
                                out=xt[:, dt, :],
                                in_=xtp[:, dt * 512:(dt + 1) * 512])
                    else:
                        nc.sync.dma_start(
                            out=xt[:],
                            in_=xtp[:, t * 4096:(t + 1) * 4096]
                            .rearrange("p (d c) -> p d c", d=8))
                    if t == 0:
                        # rope tables + AllGather-dependent load after xt(0)
                        # (in-order DMA queue; none gate the first qkv matmuls)
                        nc.sync.dma_start(out=cos_t[:], in_=cosr[:, :])
                        nc.sync.dma_start(out=sin_t[:], in_=sinr[:, :])
                        nc.sync.dma_start(
                            out=rs_full[:],
                            in_=bass_mod.AP(tensor=ag_out.ap().tensor, offset=0,
                                            ap=[[0, 128], [512, 8], [1, 512]]))
                    # per-tile rs-scaled rope tables
                    cos_rs = ph1x.tile([128, 512], BF16, tag="cosrs", bufs=3)
                    nc.vector.tensor_mul(cos_rs[:], cos_t[:, ssl],
                                         rs_full[:, t, :])
                    sin_rs = ph1x.tile([128, 512], BF16, tag="sinrs", bufs=3)
                    nc.vector.tensor_mul(sin_rs[:], sin_t[:, ssl],
                                         rs_full[:, t, :])

                    for jb in range(3):
                        pj = ph1ps.tile([128, 512], F32, tag="qkvps")
                        if QKV_F8:
                            for d in range(4):
                                nc.tensor.matmul(
                                    pj[:],
                                    wq_t[:, 2 * d:2 * d + 2,
                                         jb * 128:(jb + 1) * 128],
                                    xt[:, 2 * d:2 * d + 2, :],
                                    start=(d == 0), stop=(d == 3), perf_mode=DR)
                        else:
                            for dt in range(8):
                                nc.tensor.matmul(
                                    pj[:], wq_t[:, dt, jb * 128:(jb + 1) * 128],
                                    xt[:, dt, :], start=(dt == 0), stop=(dt == 7))
                        if jb < 2:
                            tn = (q_t, k_t)[jb]
                            qc = ph1x.tile([128, 512], BF16, tag="qc", bufs=3)
                            nc.vector.tensor_copy(qc[:], pj[:])
                            swp = ph1x.tile([128, 512], BF16, tag="swp", bufs=3)
                            nc.vector.stream_shuffle(swp[:], qc[:], SHUF_PAIR)
                            ta = ph1x.tile([128, 512], BF16, tag="ropea", bufs=3)
                            nc.vector.tensor_mul(ta[:], qc[:], cos_rs[:])
                            tb = ph1x.tile([128, 512], BF16, tag="ropeb", bufs=3)
                            nc.vector.tensor_mul(tb[:], swp[:], sin_rs[:])
                            nc.vector.tensor_add(tn[:, tsl], ta[:], tb[:])
                        else:
                            # V: scale by rs (per-token columns) while copying
                            # out of PSUM, then transpose 4 token-chunks per
                            # head into ONE PSUM tile (disjoint-column
                            # accumulate), single copy into v_tok
                            v_raw = ph1x.tile([128, 512], BF16, tag="vraw", bufs=3)
                            if QKV_F8:
                                nc.vector.scalar_tensor_tensor(
                                    v_raw[:], pj[:], 1.0 / WSCALE,
                                    rs_full[:, t, :],
                                    op0=mybir.AluOpType.mult,
                                    op1=mybir.AluOpType.mult)
                            else:
                                nc.vector.tensor_mul(v_raw[:], pj[:],
                                                     rs_full[:, t, :])
                            kt0 = (t % 4) * 4
                            for hl in range(2):
                                tp4 = ph1ps.tile([128, 4, 64], BF16, tag="trps",
                                                 bufs=1)
                                for ktl in range(4):
                                    nc.tensor.matmul(
                                        tp4[:, ktl, :],
                                        v_raw[hl * 64:(hl + 1) * 64,
                                              ktl * 128:(ktl + 1) * 128],
                                        id_t[hl * 64:(hl + 1) * 64, :],
                                        is_transpose=True,
                                        start=(ktl == 0), stop=(ktl == 3))
                                nc.vector.tensor_copy(
                                    v_tok[b][hl][:, kt0 // 2:kt0 // 2 + 2, :,
                                                 0:64], tp4[:])
                    attn_jt(0, b, t % 4)
                    if t % 4 < 3:
                        # head-1 attention for small jt interleaves here; the
                        # big (1,b1,3) tile is mostly computed at t=6 (k-tiles
                        # 0-11 are ready) with the remainder in ph2
                        attn_jt(1, b, t % 4)
                    if t == 4:
                        attn_jt(1, 0, 3)
                if debug:
                    qf = ph1b.tile([128, TOK], F32)
                    nc.vector.tensor_copy(qf[:], q_t[:])
                    nc.sync.dma_start(out=dbg["dbg_q"][:, :], in_=qf[:])
                    kf = ph1b.tile([128, TOK], F32)
                    nc.vector.tensor_copy(kf[:], k_t[:])
                    nc.sync.dma_start(out=dbg["dbg_k"][:, :], in_=kf[:])
                    vf = ph1b.tile([128, 16, 65], F32)
                    nc.vector.tensor_copy(
                        vf[:], v_tok[0][0][:, :, :, 0:65]
                        .rearrange("p a b c -> p (a b) c"))
                    nc.sync.dma_start(out=dbg["dbg_vtok"][:, :],
                                      in_=vf[:].rearrange("p a b -> p (a b)"))
            ph1ps_ctx.close()

            # ==== phase 2: second-head attention overlapped with a2a0 decode ===
            ph3 = ctx.enter_context(tc.tile_pool(name="ph3", bufs=1,
                                                 side="right"))
            ph3w = ctx.enter_context(tc.tile_pool(name="ph3w", bufs=2,
                                                  side="right"))
            ph3ps = ctx.enter_context(tc.tile_pool(name="ph3ps", bufs=2,
                                                   space="PSUM", side="right"))
            attn_n = ph3.tile([128, 8, 512], WP8, tag="attnn")
            den_t = [ph3.tile([8, 512], BF16, tag=f"den{hl}", name=f"den{hl}")
                     for hl in range(2)]
            rcpd = [ph3.tile([8, 512], F32R, tag=f"rcpd{hl}", name=f"rcpd{hl}")
                    for hl in range(2)]
            x2a = (ph3.tile([128, 8, 512], F32R, tag="x2a")
                   if SPLITPROJ else None)
            x2 = ph3.tile([128, 8, 512], F32R, tag="x2")

            def decode_half(hl):
                """load a2a_out[hl] + normalize by the softmax denominators."""
                nc.sync.dma_start(
                    out=den_t[hl][:],
                    in_=a2a_out[hl][:, 64:65, :].rearrange("i f t -> (i f) t"))
                nc.sync.dma_start(
                    out=attn_n[hl * 64:(hl + 1) * 64, :, :],
                    in_=a2a_out[hl][:, 0:64, :].rearrange("i f t -> f i t"))
                with nc.allow_low_precision(reason="denom recip f32r"):
                    nc.vector.reciprocal(rcpd[hl][:], den_t[hl][:])
                sl = slice(hl * 64, (hl + 1) * 64)
                for dt in range(8):
                    rcpb = ph3ps.tile([128, 512], F32, tag="pp")
                    nc.tensor.matmul(rcpb[:], sel_t[:, dt, :], rcpd[hl][:],
                                     start=True, stop=True)
                    nc.vector.tensor_mul(attn_n[sl, dt, :], attn_n[sl, dt, :],
                                         rcpb[sl, :])

            def proj_half(hl, acc_in, acc_out):
                sl = slice(hl * 64, (hl + 1) * 64)
                for jb in range(8):
                    pp = ph3ps.tile([128, 512], F32, tag="pp")
                    for dt in range(8):
                        nc.tensor.matmul(pp[:], wpj_t[sl, jb, dt, :],
                                         attn_n[sl, dt, :],
                                         start=(dt == 0), stop=(dt == 7))
                    nc.vector.tensor_add(acc_out[:, jb, :], pp[:],
                                         acc_in[:, jb, :])

            def proj_full():
                for jb in range(8):
                    pp = ph3ps.tile([128, 512], F32, tag="pp")
                    for dt in range(8):
                        nc.tensor.matmul(pp[:], wpj_t[:, jb, dt, :],
                                         attn_n[:, dt, :],
                                         start=(dt == 0), stop=(dt == 7))
                    nc.vector.tensor_add(x2[:, jb, :], pp[:],
                                         xown.bitcast(F32)[:, jb, :])

            with nc.named_scope("ph2_attn1_a2a"):
                if sim:
                    nc.sync.dma_start(out=a2a_out[0].ap(), in_=a2a_in[0].ap())
                else:
                    nc.gpsimd.collective_compute(
                        "AllToAll", mybir.AluOpType.bypass, replica_groups=RG,
                        ins=[a2a_in[0].ap().opt()], outs=[a2a_out[0].ap().opt()])
                attn_jt(1, 1, 3)
                nc.sync.dma_start(out=wpj_t[:],
                                  in_=wproj[:, :].rearrange(
                                      "p (j d c) -> p j d c", j=8, d=8))
                decode_half(0)
                if SPLITPROJ:
                    proj_half(0, xown.bitcast(F32), x2a)
                if sim:
                    nc.sync.dma_start(out=a2a_out[1].ap(), in_=a2a_in[1].ap())
                else:
                    nc.gpsimd.collective_compute(
                        "AllToAll", mybir.AluOpType.bypass, replica_groups=RG,
                        ins=[a2a_in[1].ap().opt()], outs=[a2a_out[1].ap().opt()])
                decode_half(1)
                if SPLITPROJ:
                    proj_half(1, x2a.bitcast(F32), x2)
                else:
                    proj_full()
            qkctx.close()  # free Q/K/V + attention SBUF/PSUM before the FFN phase

            # ============ phase 3: LN2 + FFN ==================================
            with tc.tile_pool(name="ph3st", bufs=2, space="PSUM") as ph3st:
                ph3scope = contextlib.ExitStack()
                ph3scope.enter_context(nc.named_scope("ph3_proj_ln2"))
                if debug:
                    atf = ph3w.tile([128, 8, 512], F32, tag="dbgst", bufs=1)
                    nc.vector.tensor_copy(atf[:], attn_n[:])
                    nc.sync.dma_start(out=dbg["dbg_att"][:, :],
                                      in_=atf[:].rearrange("p a b -> p (a b)"))
                if debug:
                    nc.sync.dma_start(out=dbg["dbg_x2"][:, :],
                                      in_=x2.bitcast(F32)[:]
                                      .rearrange("p a b -> p (a b)"))

                wc1 = None
                if not FC_F8:
                    wc1 = ph3w.tile([128, 4, 8, 128], WF8, tag="wc1")
                    nc.sync.dma_start(
                        out=wc1[:],
                        in_=wfc1[:, 0:4096]
                        .rearrange("p (g d c) -> p g d c", g=4, d=8))
                # LN2 stats (local, own tokens); ones-matmul broadcasts across
                # all 128 partitions for free
                mps2 = ph3st.tile([128, 512], F32, tag="stats")
                sps2 = ph3st.tile([128, 512], F32, tag="stats")
                for dt in range(8):
                    nc.tensor.matmul(mps2[:], ones_t[:], x2[:, dt, :],
                                     start=(dt == 0), stop=(dt == 7))
                for dt in range(8):
                    sq2 = ph3w.tile([128, 512], F32R, tag="sq2", bufs=3)
                    nc.vector.tensor_mul(sq2[:], x2.bitcast(F32)[:, dt, :],
                                         x2.bitcast(F32)[:, dt, :])
                    nc.tensor.matmul(sps2[:], ones_t[:], sq2[:],
                                     start=(dt == 0), stop=(dt == 7))
                var2 = ph3.tile([128, 512], F32)
                nc.scalar.square(var2[:], mps2[:])
                nc.vector.tensor_sub(var2[:], sps2[:], var2[:])
                srt2 = ph3.tile([128, 512], F32)
                nc.scalar.activation(srt2[:], var2[:], AF.Sqrt, bias=eps128[:])
                rstd = ph3.tile([128, 512], BF16)
                with nc.allow_low_precision(reason="ln2 rstd bf16"):
                    nc.vector.reciprocal(rstd[:], srt2[:])
                if debug:
                    m2row = ph3w.tile([1, 512], F32, tag="m2row", bufs=1)
                    nc.vector.tensor_copy(m2row[:], mps2[0:1, :])
                    rstdf = ph3w.tile([1, 512], F32, tag="m2row", bufs=1)
                    nc.vector.tensor_copy(rstdf[:], rstd[0:1, :])
                    nc.sync.dma_start(out=dbg["dbg_rs2"][:, :], in_=rstdf[:])
                    nc.sync.dma_start(out=dbg["dbg_m2"][:, :], in_=m2row[:])
                x2n = ph3.tile([128, 8, 512], WF8)
                for dt in range(8):
                    tnrm = ph3w.tile([128, 512], BF16, tag="tnrm")
                    nc.vector.tensor_sub(tnrm[:], x2.bitcast(F32)[:, dt, :],
                                         mps2[:])
                    nc.vector.tensor_mul(x2n[:, dt, :], tnrm[:], rstd[:])

                ph3scope.close()
                ph3scope = contextlib.ExitStack()
                ph3scope.enter_context(nc.named_scope("ph3_fc1"))
                # fc1 + gelu (descale folded into activation scale)
                h1 = ph3.tile([128, 32, 512], WF8)
                gsc = 1.0 / WSCALE if FC_F8 else 1.0
                for j in range(32):
                    pf = ph3st.tile([128, 512], F32, tag="pf")
                    if FC_F8:
                        for d in range(4):
                            nc.tensor.matmul(
                                pf[:], wf1_t[:, j, 2 * d:2 * d + 2, :],
                                x2n[:, 2 * d:2 * d + 2, :],
                                start=(d == 0), stop=(d == 3), perf_mode=DR)
                    else:
                        if j % 4 == 0 and j > 0:
                            wc1 = ph3w.tile([128, 4, 8, 128], WF8, tag="wc1")
                            nc.sync.dma_start(
                                out=wc1[:],
                                in_=wfc1[:, j * 1024:(j + 4) * 1024]
                                .rearrange("p (g d c) -> p g d c", g=4, d=8))
                        for dt in range(8):
                            nc.tensor.matmul(pf[:], wc1[:, j % 4, dt, :],
                                             x2n[:, dt, :], start=(dt == 0),
                                             stop=(dt == 7))
                    nc.scalar.activation(h1[:, j, :], pf[:], AF.Gelu, scale=gsc)
                if debug:
                    h1f = ph3w.tile([128, 4, 512], F32, tag="dbgst", bufs=1)
                    nc.vector.tensor_copy(h1f[:], h1[:, 0:4, :])
                    nc.sync.dma_start(out=dbg["dbg_h1"][:, :],
                                      in_=h1f[:].rearrange("p a b -> p (a b)"))

                ph3scope.close()
                ph3scope.enter_context(nc.named_scope("ph3_fc2"))
                # fc2 + residual + out
                for d in range(8):
                    pf2 = ph3st.tile([128, 512], F32, tag="pf")
                    for half in range(2):
                        wc2 = ph3w.tile([128, 16, 128], WF8, tag="wc")
                        nc.sync.dma_start(
                            out=wc2[:],
                            in_=wfc2[:, (d * 32 + half * 16) * 128:
                                     (d * 32 + (half + 1) * 16) * 128]
                            .rearrange("p (j c) -> p j c", j=16))
                        if FC_F8:
                            for jj in range(8):
                                jp = half * 8 + jj
                                nc.tensor.matmul(
                                    pf2[:], wc2[:, 2 * jj:2 * jj + 2, :],
                                    h1[:, 2 * jp:2 * jp + 2, :],
                                    start=(jp == 0), stop=(jp == 15),
                                    perf_mode=DR)
                        else:
                            for jj in range(16):
                                jt = half * 16 + jj
                                nc.tensor.matmul(pf2[:], wc2[:, jj, :],
                                                 h1[:, jt, :], start=(jt == 0),
                                                 stop=(jt == 31))
                    ot = ph3w.tile([128, 512], F32, tag="ot")
                    if FC_F8:
                        nc.vector.scalar_tensor_tensor(
                            ot[:], pf2[:], 1.0 / WSCALE, x2.bitcast(F32)[:, d, :],
                            op0=mybir.AluOpType.mult, op1=mybir.AluOpType.add)
                    else:
                        nc.vector.tensor_add(ot[:], pf2[:],
                                             x2.bitcast(F32)[:, d, :])
                    nc.sync.dma_start(out=outT[:, d * 512:(d + 1) * 512], in_=ot[:])
                ph3scope.close()

    nc.compile()
    _BUILD_CACHE[key] = nc
    return nc


def _prep_inputs(x, sin, cos, ln1_w, w_qkv, w_proj, ln2_w, w_fc1, w_fc2):
    """Host-side packing/folding. Returns in_maps (list of 8 dicts)."""
    import ml_dtypes
    F8NP = ml_dtypes.float8_e4m3
    BF16NP = ml_dtypes.bfloat16
    W8NP = F8NP if QKV_F8 else BF16NP
    WP8NP = F8NP if PROJ_F8 else BF16NP
    WF8NP = F8NP if FC_F8 else BF16NP
    PT8NP = F8NP if PV_F8 else BF16NP

    xf = np.ascontiguousarray(x.reshape(TOK, D).T)          # [1024, 4096]
    xtp_r = round_fp32r(xf)
    xw8 = xf.astype(W8NP)
    xtp = np.empty((128, 8 * 8 * 512), W8NP)
    for t in range(8):
        for dt in range(8):
            xtp[:, (t * 8 + dt) * 512:(t * 8 + dt + 1) * 512] = \
                xw8[dt * 128:(dt + 1) * 128, t * 512:(t + 1) * 512]

    # qkv weight fold: ln1_w scale, q-scale 1/8, mean-subtraction fold,
    # pair-interleaved head-dim permutation for q/k, fp8 pre-scale
    w1 = (ln1_w[:, None] * w_qkv).astype(np.float64)
    w1[:, :D] *= 1.0 / np.sqrt(HD)
    w1 = w1 - w1.mean(axis=0, keepdims=True)
    if QKV_F8:
        w1 = w1 * WSCALE
    w1 = w1.astype(np.float32)

    # head-dim permutation: new pos p <- old dim p//2 + 32*(p%2)
    perm = np.array([p // 2 + 32 * (p % 2) for p in range(HD)])

    wp = w_proj.astype(np.float64)
    if PROJ_F8:
        wp = wp * WSCALE
    wp = wp.astype(np.float32).astype(WP8NP)
    wproj_p = np.empty((128, 8 * 8 * 128), WP8NP)
    for jb in range(8):
        for dt in range(8):
            wproj_p[:, (jb * 8 + dt) * 128:(jb * 8 + dt + 1) * 128] = \
                wp[dt * 128:(dt + 1) * 128, jb * 128:(jb + 1) * 128]

    wf1 = (ln2_w[:, None] * w_fc1).astype(np.float64)
    if FC_F8:
        wf1 = wf1 * WSCALE
    wf1 = wf1.astype(np.float32).astype(WF8NP)               # [1024, 4096]
    wfc1_p = np.empty((128, 32 * 8 * 128), WF8NP)
    for j in range(32):
        for dt in range(8):
            wfc1_p[:, (j * 8 + dt) * 128:(j * 8 + dt + 1) * 128] = \
                wf1[dt * 128:(dt + 1) * 128, j * 128:(j + 1) * 128]
    wf2 = w_fc2.astype(np.float64)
    if FC_F8:
        wf2 = wf2 * WSCALE
    wf2 = wf2.astype(np.float32).astype(WF8NP)               # [4096, 1024]
    wfc2_p = np.empty((128, 8 * 32 * 128), WF8NP)
    for d in range(8):
        for jt in range(32):
            wfc2_p[:, (d * 32 + jt) * 128:(d * 32 + jt + 1) * 128] = \
                wf2[jt * 128:(jt + 1) * 128, d * 128:(d + 1) * 128]

    # rope tables in pair-interleaved order; sin sign-folded (even rows -),
    # fp8 descale folded in when QKV_F8
    cos1 = np.ascontiguousarray(cos.reshape(S, HD).T, dtype=np.float64)  # [64,S]
    sin1 = np.ascontiguousarray(sin.reshape(S, HD).T, dtype=np.float64)
    cosp = cos1[perm, :].copy()
    sinp = sin1[perm, :].copy()
    sinp[0::2, :] *= -1.0
    if QKV_F8:
        cosp /= WSCALE
        sinp /= WSCALE
    cosf = np.concatenate([cosp, cosp], axis=0).astype(BF16NP)  # [128, S]
    sinT = np.concatenate([sinp, sinp], axis=0).astype(BF16NP)

    maskt = np.zeros((128, 4, 512), np.float32)
    rk = np.arange(128)[:, None]
    rq = np.arange(512)[None, :]
    for o in range(4):
        valid = (128 * o + rk <= rq)
        maskt[:, o, :] = (np.where(valid, 0.0, -60.0) if PV_F8
                          else valid.astype(np.float32))
    maskt = maskt.reshape(128, 4 * 512).astype(BF16NP)
    id128 = np.eye(128, dtype=np.float32).astype(BF16NP)

    ones128 = np.full((128, 128), 1.0 / D, np.float32)
    ident2 = np.concatenate([np.eye(64, dtype=np.float32)] * 2,
                            axis=0).astype(BF16NP)
    # denom broadcast one-hots: selt[r, dt*128+p] = (r == dt)
    selt = np.zeros((8, 8, 128), np.float32)
    for dt in range(8):
        selt[dt, dt, :] = 1.0
    selt = selt.reshape(8, 8 * 128)

    in_maps = []
    for c in range(NCORES):
        heads = [2 * c, 2 * c + 1]
        cols = []
        for blk in range(3):  # Q, K, V
            for h in heads:
                base = blk * D + h * HD
                if blk < 2:
                    cols.extend(base + perm)
                else:
                    cols.extend(range(base, base + HD))
        wsel = w1[:, cols].astype(W8NP)  # [1024, 384]
        wqkv_p = np.empty((128, 8 * 384), W8NP)
        for dt in range(8):
            wqkv_p[:, dt * 384:(dt + 1) * 384] = wsel[dt * 128:(dt + 1) * 128, :]
        xres = np.empty((128, 8 * 512), np.float32)
        xslice = xtp_r[:, c * TOWN:(c + 1) * TOWN]  # [1024, 512] pre-rounded
        for dt in range(8):
            xres[:, dt * 512:(dt + 1) * 512] = xslice[dt * 128:(dt + 1) * 128, :]
        in_maps.append({
            "xtp": xtp, "xres": xres, "wqkv": wqkv_p, "wproj": wproj_p,
            "wfc1": wfc1_p, "wfc2": wfc2_p, "cosr": cosf, "sinr": sinT,
            "maskt": maskt, "ones128": ones128, "ident2": ident2,
            "id128": id128, "selt": selt,
        })
    return in_maps


def _assemble_output(results):
    full = np.empty((TOK, D), np.float32)
    for c in range(NCORES):
        blk = results[c]["outT"].reshape(128, 8, 512)
        for d in range(8):
            full[c * TOWN:(c + 1) * TOWN, d * 128:(d + 1) * 128] = blk[:, d, :].T
    return full.reshape(B, S, D)


def kernel(x, mask, sin, cos, ln1_w, ln1_b, w_qkv, w_proj, ln2_w, ln2_b,
           w_fc1, w_fc2):
    x = np.asarray(x, np.float32)
    mask_np = np.asarray(mask)
    causal = np.array_equal(
        mask_np.reshape(S, S), np.tril(np.ones((S, S), dtype=bool)))
    biases_zero = (np.abs(np.asarray(ln1_b)).max() == 0.0 and
                   np.abs(np.asarray(ln2_b)).max() == 0.0)
    if not (causal and biases_zero):
        return _np_reference(x, mask_np, np.asarray(sin), np.asarray(cos),
                             np.asarray(ln1_w), np.asarray(ln1_b),
                             np.asarray(w_qkv), np.asarray(w_proj),
                             np.asarray(ln2_w), np.asarray(ln2_b),
                             np.asarray(w_fc1), np.asarray(w_fc2))

    import jax
    try:
        jax.config.update("jax_compilation_cache_dir", "/tmp/jax_nc_cache")
        jax.config.update("jax_persistent_cache_min_compile_time_secs", 0.0)
        jax.config.update("jax_persistent_cache_min_entry_size_bytes", 0)
    except Exception:
        pass
    from concourse.bass_utils import run_bass_kernel_spmd
    nc = _build(debug=False)
    in_maps = _prep_inputs(x, np.asarray(sin, np.float32).reshape(S, HD),
                           np.asarray(cos, np.float32).reshape(S, HD),
                           np.asarray(ln1_w, np.float32),
                           np.asarray(w_qkv, np.float32),
                           np.asarray(w_proj, np.float32),
                           np.asarray(ln2_w, np.float32),
                           np.asarray(w_fc1, np.float32),
                           np.asarray(w_fc2, np.float32))
    res = run_bass_kernel_spmd(nc, in_maps, core_ids=list(range(NCORES)))
    return _assemble_output(res.results)

